# revision 21
# baseline (speedup 1.0000x reference)
"""Multi-head attention (B=8, N=1024, D=768, H=12) on 8 TRN2 NeuronCores.

Sharding: data-parallel over batch — core b computes batch element b.

Per-core kernel (all shapes hardcoded):
  inputs (host-prepped):
    xT   (768, 1024) f32  = x[b].T
    w    (768, 2304) f32  = W_qkv
    b_qk (128, 12)   f32  = b_qkv[:1536] laid out [partition, chunk]
    b_v  (1, 768)    f32  = b_qkv[1536:]
  output:
    out  (1024, 768) f32

Pipeline (single fully-interleaved phase to keep TensorE dense and the
HAM clock at 8/8):
  - qkT chunks ((x @ W_qk + b)^T, fp32r matmuls) are produced one head
    pair ahead of the S matmuls that consume them.
  - v = x @ W_v + b_v is computed during the first pair's S steps and
    stored per head as bf16 [v_hi 64 | ones 1 | v_lo 64 | pad 1] blocks
    (hi/lo split keeps v exact; ones column gives the softmax denominator).
  - S^T[j,i] = k @ q^T per head pair via row-tiled K=64 matmuls (two heads
    concurrently on PE row groups 0:64 / 64:128).
  - E = exp(S^T) -> bf16 on ScalarE straight from PSUM (no max subtraction:
    logits are bounded ~ +-50 here, exp stays in f32/bf16 range).
  - PV: psum[i-block, 0:130] = sum_j E^T[j] @ v_block (bf16, FWL); PV
    i-steps of the previous pair fill PE slack between S j-steps.
  - epilogue per (head, i): out = (hi + lo) * recip(denom) on DVE, DMA out.
"""

from collections import deque

import numpy as np

import concourse.bass as bass
import concourse.mybir as mybir
import concourse.tile as tile
from concourse import bacc
from concourse.bass_utils import run_bass_kernel_spmd

N_CORES = 8
NSEQ = 1024
DMODEL = 768
H = 12
DH = 64
C3 = 3 * DMODEL
KC = DMODEL // 128   # 6 contraction chunks
MI = NSEQ // 128     # 8 sequence chunks
VB = 2 * DH + 2      # 130: per-head v block [hi 64 | ones 1 | lo 64 | pad 1]

F32 = mybir.dt.float32
F32R = mybir.dt.float32r
BF16 = mybir.dt.bfloat16
EXP = mybir.ActivationFunctionType.Exp
MUL = mybir.AluOpType.mult
ADD = mybir.AluOpType.add

_NC_CACHE = {}


def build_nc(with_bias=True):
    key = ("nc", with_bias)
    if key in _NC_CACHE:
        return _NC_CACHE[key]
    nc = bacc.Bacc("TRN2", target_bir_lowering=False, debug=False)
    xa_d = nc.dram_tensor("xa", [KC, 128, 512], F32R, kind="ExternalInput")
    xb_d = nc.dram_tensor("xb", [KC, 128, 512], F32R, kind="ExternalInput")
    wp_d = nc.dram_tensor("wp", [KC, 128, KC * 256], F32R, kind="ExternalInput")
    wv_d = nc.dram_tensor("wv", [KC, 128, DMODEL], F32R, kind="ExternalInput")
    bqk_d = nc.dram_tensor("b_qk", [128, 2 * KC], F32, kind="ExternalInput")
    bv_d = nc.dram_tensor("b_v", [1, DMODEL], F32R, kind="ExternalInput")
    ones_d = nc.dram_tensor("ones_in", [1, 128], F32R, kind="ExternalInput")
    out_d = nc.dram_tensor("out", [NSEQ, DMODEL], F32, kind="ExternalOutput")

    with tile.TileContext(nc) as tc:
        with (
            tc.tile_pool(name="const", bufs=1) as cpool,
            tc.tile_pool(name="main", bufs=1) as mpool,
            tc.tile_pool(name="stage", bufs=8) as stpool,
            tc.tile_pool(name="e", bufs=32) as epool,
            tc.tile_pool(name="wt", bufs=3) as wpool,
            tc.tile_pool(name="qkt", bufs=8) as qkpool,
            tc.tile_pool(name="s_ps", bufs=3, space="PSUM") as sps,
            tc.tile_pool(name="mix_ps", bufs=2, space="PSUM") as mps,
        ):
            b_qk = cpool.tile([128, 2 * KC], F32, tag="bqk")
            nc.sync.dma_start(b_qk[:], bqk_d[:])
            b_v = cpool.tile([1, DMODEL], F32R, tag="bv")
            nc.sync.dma_start(b_v[:], bv_d[:])
            ones1 = cpool.tile([1, 128], F32R, tag="ones")
            nc.sync.dma_start(ones1[:], ones_d[:])

            # persistent activations
            v_ext = [mpool.tile([128, H * VB], BF16, tag=f"vx{j}", name=f"vx{j}")
                     for j in range(MI)]
            # x^T halves, per k-chunk
            xT_a = [mpool.tile([128, 512], F32R, tag=f"xa{k}", name=f"xa{k}")
                    for k in range(KC)]
            xT_b = [mpool.tile([128, 512], F32R, tag=f"xb{k}", name=f"xb{k}")
                    for k in range(KC)]

            # W_q/W_k packed per head pair: tile[:, k, 0:128] = q chunk cols,
            # tile[:, k, 128:256] = k chunk cols. One contiguous DMA per pair.
            w_t = {}

            def load_w(pm):
                t = wpool.tile([128, KC * 256], F32R, tag="w", name=f"wp{pm}")
                nc.sync.dma_start(t[:], wp_d[pm])
                w_t[pm] = t

            # qkT chunks from a recycled pool (live: current + next pair)
            qkt = {}

            load_w(0)
            for k in range(KC):
                nc.sync.dma_start(xT_a[k][:], xa_d[k])
                nc.sync.dma_start(xT_b[k][:], xb_d[k])

            with tc.tile_pool(name="wv", bufs=1) as wvpool:
                w_v = [wvpool.tile([128, DMODEL], F32R, tag=f"wv{k}", name=f"wv{k}")
                       for k in range(KC)]
                for k in range(KC):
                    nc.sync.dma_start(w_v[k][:], wv_d[k])
                # remaining W_q/W_k pair blocks, in consumption order
                for m in range(1, KC):
                    load_w(m)

                xhalf = [xT_a, xT_b]

                def qk_chunk(mm, n):
                    if n == 0:
                        qkt[mm] = qkpool.tile(
                            [128, NSEQ], F32R, tag="qkt", name=f"qkt{mm}")
                    ps = mps.tile([128, 512], F32, tag="mps", name="ps_qk")
                    off = 0 if mm < KC else 128
                    w3 = w_t[mm % KC].rearrange("p (k c) -> p k c", c=256)
                    for k in range(KC):
                        nc.tensor.matmul(
                            ps[:],
                            lhsT=w3[:, k, off:off + 128],
                            rhs=xhalf[n][k][:],
                            start=(k == 0), stop=(k == KC - 1),
                        )
                    nc.vector.tensor_scalar_add(
                        qkt[mm][:, n * 512:(n + 1) * 512], ps[:], b_qk[:, mm:mm + 1],
                    )

                def v_chunk(mi, n0, nw):
                    ps = mps.tile([128, 512], F32, tag="mps", name="ps_v")
                    xh = xhalf[mi // 4]
                    c0 = (mi % 4) * 128
                    for k in range(KC):
                        nc.tensor.matmul(
                            ps[:, :nw],
                            lhsT=xh[k][:, c0:c0 + 128],
                            rhs=w_v[k][:, n0:n0 + nw],
                            start=(k == 0), stop=(with_bias is False and k == KC - 1),
                        )
                    if with_bias:
                        nc.tensor.matmul(
                            ps[:, :nw], lhsT=ones1[:, :],
                            rhs=b_v[:, n0:n0 + nw], start=False, stop=True,
                        )
                    nh = nw // DH
                    h0 = n0 // DH
                    src = ps[:, :nw].rearrange("p (h c) -> p h c", c=DH)
                    dst3 = v_ext[mi].rearrange("p (h c) -> p h c", c=VB)
                    hi = dst3[:, h0:h0 + nh, 0:DH]
                    lo = dst3[:, h0:h0 + nh, DH + 1:DH + 1 + DH]
                    nc.vector.tensor_copy(hi, src)
                    nc.vector.tensor_sub(lo, src, hi)

                for mi in range(MI):
                    d3 = v_ext[mi].rearrange("p (h c) -> p h c", c=VB)
                    nc.vector.memset(d3[:, :, DH:DH + 1], 1.0)
                    nc.vector.memset(d3[:, :, VB - 1:VB], 0.0)

                pvq = deque()  # deferred PV i-steps: (head, i, E tiles)

                def pv_step(h, i, E0, E1):
                    # E0[j] = [A cols 0:512 | B cols 0:512] of S^T row-block j,
                    # E1[j] = the 512:1024 column halves
                    off = 512 * (h % 2)
                    Ei = E0 if i < 4 else E1
                    c0 = off + (i % 4) * 128
                    pv = mps.tile([128, VB], F32, tag="mps", name="pv")
                    for j in range(MI):
                        nc.tensor.matmul(
                            pv[:, :VB],
                            lhsT=Ei[j][:, c0:c0 + 128],
                            rhs=v_ext[j][:, h * VB:(h + 1) * VB],
                            start=(j == 0), stop=(j == MI - 1),
                        )
                    r = stpool.tile([128, 1], F32, tag="r", name="r")
                    nc.vector.reciprocal(r[:], pv[:, DH:DH + 1])
                    u = stpool.tile([128, DH], F32, tag="u", name="u")
                    nc.vector.tensor_scalar(u[:], pv[:, 0:DH], r[:], None, op0=MUL)
                    o = stpool.tile([128, DH], F32, tag="o", name="o")
                    nc.vector.scalar_tensor_tensor(
                        o[:], pv[:, DH + 1:DH + 1 + DH], r[:], u[:],
                        op0=MUL, op1=ADD)
                    nc.sync.dma_start(
                        out_d[i * 128:(i + 1) * 128, h * DH:(h + 1) * DH], o[:],
                    )

                # prologue: qkT chunks for pair 0
                for mm in (0, KC):
                    for n in range(2):
                        qk_chunk(mm, n)

                vq = deque((mi, n0, nw) for mi in range(MI)
                           for n0, nw in ((0, 512), (512, 256)))

                for pm in range(H // 2):
                    hA, hB = 2 * pm, 2 * pm + 1
                    q_t, k_t = qkt[pm], qkt[KC + pm]
                    EA, EB = [], []
                    nxt = []
                    if pm + 1 < H // 2:
                        nxt = [(pm + 1, 0), (pm + 1, 1),
                               (KC + pm + 1, 0), (KC + pm + 1, 1)]
                    for j in range(MI):
                        # S j-step: A and B share each slab (A -> left bank,
                        # B -> right bank) so one exp releases both heads'
                        # next matmuls and the row-tiled pair runs in parallel
                        ps0 = sps.tile([128, NSEQ], F32, tag="sps", name="ps0")
                        ps1 = sps.tile([128, NSEQ], F32, tag="sps", name="ps1")
                        with tc.high_priority(offset=200000):
                            for n, psn in ((0, ps0), (1, ps1)):
                                nc.tensor.matmul(
                                    psn[:, 0:512],
                                    lhsT=k_t[0:64, j * 128:(j + 1) * 128],
                                    rhs=q_t[0:64, n * 512:(n + 1) * 512],
                                    start=True, stop=True, tile_position=(0, 0),
                                )
                                nc.tensor.matmul(
                                    psn[:, 512:1024],
                                    lhsT=k_t[64:128, j * 128:(j + 1) * 128],
                                    rhs=q_t[64:128, n * 512:(n + 1) * 512],
                                    start=True, stop=True, tile_position=(64, 0),
                                )
                        e0 = epool.tile([128, NSEQ], BF16, tag="e", name="e0")
                        e1 = epool.tile([128, NSEQ], BF16, tag="e", name="e1")
                        nc.scalar.activation(e0[:], ps0[:], EXP)
                        nc.scalar.activation(e1[:], ps1[:], EXP)
                        EA.append(e0)
                        EB.append(e1)
                        # fill work after the S pair: lower scheduler priority,
                        # so it runs only while S matmuls are stalled
                        if pm == 0:
                            for _ in range(2):
                                if vq:
                                    v_chunk(*vq.popleft())
                        if j % 2 == 0 and nxt:
                            qk_chunk(*nxt.pop(0))
                        for _ in range(2):
                            if pvq:
                                pv_step(*pvq.popleft())
                    pvq.extend((hA, i, EA, EB) for i in range(MI))
                    pvq.extend((hB, i, EA, EB) for i in range(MI))
                while pvq:
                    pv_step(*pvq.popleft())

    nc.compile()
    _NC_CACHE[key] = nc
    return nc


def make_in_maps(x, W_qkv, b_qkv):
    x = np.asarray(x, dtype=np.float32)
    W_qkv = np.asarray(W_qkv, dtype=np.float32)
    b_qkv = np.asarray(b_qkv, dtype=np.float32)
    xT = x.transpose(0, 2, 1)                                # (B, 768, 1024)
    xa = np.ascontiguousarray(
        xT[:, :, 0:512].reshape(N_CORES, KC, 128, 512))
    xb = np.ascontiguousarray(
        xT[:, :, 512:1024].reshape(N_CORES, KC, 128, 512))
    # wp[pm] = [128 part, KC, 256] with q-chunk cols then k-chunk cols
    wr = W_qkv.reshape(KC, 128, C3)
    blocks = []
    for pm in range(KC):
        qp = wr[:, :, pm * 128:(pm + 1) * 128]               # (KC, 128, 128)
        kp = wr[:, :, DMODEL + pm * 128:DMODEL + (pm + 1) * 128]
        blocks.append(np.concatenate([qp, kp], axis=2)       # (KC, 128, 256)
                      .transpose(1, 0, 2))                   # (128, KC, 256)
    wp = np.ascontiguousarray(
        np.stack(blocks).reshape(KC, 128, KC * 256))
    wv = np.ascontiguousarray(wr[:, :, 2 * DMODEL:C3])       # (KC, 128, 768)
    b_qk = np.ascontiguousarray(
        b_qkv[:2 * DMODEL].reshape(2 * KC, 128).T)           # (128, 12)
    b_v = np.ascontiguousarray(b_qkv[2 * DMODEL:].reshape(1, DMODEL))
    ones_in = np.ones((1, 128), dtype=np.float32)
    return [
        {"xa": xa[c], "xb": xb[c], "wp": wp, "wv": wv,
         "b_qk": b_qk, "b_v": b_v, "ones_in": ones_in}
        for c in range(N_CORES)
    ]


def run(in_maps, trace=False, trace_cores=None, with_bias=True):
    nc = build_nc(with_bias=with_bias)
    return run_bass_kernel_spmd(
        nc, in_maps, list(range(N_CORES)), trace=trace, trace_cores=trace_cores,
    )


def kernel(x, W_qkv, b_qkv):
    with_bias = bool(np.any(np.asarray(b_qkv)))
    res = run(make_in_maps(x, W_qkv, b_qkv), with_bias=with_bias)
    return np.stack([res.results[c]["out"] for c in range(N_CORES)]).astype(np.float32)


# revision 22
# speedup vs baseline: 1.0937x; 1.0937x over previous
"""Multi-head attention (B=8, N=1024, D=768, H=12) on 8 TRN2 NeuronCores.

Sharding: data-parallel over batch — core b computes batch element b.

Per-core kernel (all shapes hardcoded):
  inputs (host-prepped):
    xT   (768, 1024) f32  = x[b].T
    w    (768, 2304) f32  = W_qkv
    b_qk (128, 12)   f32  = b_qkv[:1536] laid out [partition, chunk]
    b_v  (1, 768)    f32  = b_qkv[1536:]
  output:
    out  (1024, 768) f32

Pipeline (single fully-interleaved phase to keep TensorE dense and the
HAM clock at 8/8):
  - qkT chunks ((x @ W_qk + b)^T, fp32r matmuls) are produced one head
    pair ahead of the S matmuls that consume them.
  - v = x @ W_v + b_v is computed during the first pair's S steps and
    stored per head as bf16 [v_hi 64 | ones 1 | v_lo 64 | pad 1] blocks
    (hi/lo split keeps v exact; ones column gives the softmax denominator).
  - S^T[j,i] = k @ q^T per head pair via row-tiled K=64 matmuls (two heads
    concurrently on PE row groups 0:64 / 64:128).
  - E = exp(S^T) -> bf16 on ScalarE straight from PSUM (no max subtraction:
    logits are bounded ~ +-50 here, exp stays in f32/bf16 range).
  - PV: psum[i-block, 0:130] = sum_j E^T[j] @ v_block (bf16, FWL); PV
    i-steps of the previous pair fill PE slack between S j-steps.
  - epilogue per (head, i): out = (hi + lo) * recip(denom) on DVE, DMA out.
"""

from collections import deque

import numpy as np

import concourse.bass as bass
import concourse.mybir as mybir
import concourse.tile as tile
from concourse import bacc
from concourse.bass_utils import run_bass_kernel_spmd

N_CORES = 8
NSEQ = 1024
DMODEL = 768
H = 12
DH = 64
C3 = 3 * DMODEL
KC = DMODEL // 128   # 6 contraction chunks
MI = NSEQ // 128     # 8 sequence chunks
VB = 2 * DH + 2      # 130: per-head v block [hi 64 | ones 1 | lo 64 | pad 1]

F32 = mybir.dt.float32
F32R = mybir.dt.float32r
BF16 = mybir.dt.bfloat16
EXP = mybir.ActivationFunctionType.Exp
MUL = mybir.AluOpType.mult
ADD = mybir.AluOpType.add

_NC_CACHE = {}


def build_nc(with_bias=True):
    key = ("nc", with_bias)
    if key in _NC_CACHE:
        return _NC_CACHE[key]
    nc = bacc.Bacc("TRN2", target_bir_lowering=False, debug=False)
    xa_d = nc.dram_tensor("xa", [KC, 128, 512], F32R, kind="ExternalInput")
    xb_d = nc.dram_tensor("xb", [KC, 128, 512], F32R, kind="ExternalInput")
    wp_d = nc.dram_tensor("wp", [KC, 128, KC * 256], F32R, kind="ExternalInput")
    wv_d = nc.dram_tensor("wv", [KC, 128, DMODEL], F32R, kind="ExternalInput")
    bqk_d = nc.dram_tensor("b_qk", [128, 2 * KC], F32, kind="ExternalInput")
    bv_d = nc.dram_tensor("b_v", [1, DMODEL], F32R, kind="ExternalInput")
    ones_d = nc.dram_tensor("ones_in", [1, 128], F32R, kind="ExternalInput")
    out_d = nc.dram_tensor("out", [NSEQ, DMODEL], F32, kind="ExternalOutput")

    with tile.TileContext(nc) as tc:
        with (
            tc.tile_pool(name="const", bufs=1) as cpool,
            tc.tile_pool(name="main", bufs=1) as mpool,
            tc.tile_pool(name="stage", bufs=8) as stpool,
            tc.tile_pool(name="e", bufs=32) as epool,
            tc.tile_pool(name="wt", bufs=3) as wpool,
            tc.tile_pool(name="qkt", bufs=8) as qkpool,
            tc.tile_pool(name="s_ps", bufs=3, space="PSUM") as sps,
            tc.tile_pool(name="mix_ps", bufs=2, space="PSUM") as mps,
        ):
            b_qk = cpool.tile([128, 2 * KC], F32, tag="bqk")
            nc.sync.dma_start(b_qk[:], bqk_d[:])
            b_v = cpool.tile([1, DMODEL], F32R, tag="bv")
            nc.sync.dma_start(b_v[:], bv_d[:])
            ones1 = cpool.tile([1, 128], F32R, tag="ones")
            nc.sync.dma_start(ones1[:], ones_d[:])

            # persistent activations
            v_ext = [mpool.tile([128, H * VB], BF16, tag=f"vx{j}", name=f"vx{j}")
                     for j in range(MI)]
            # x^T halves, per k-chunk
            xT_a = [mpool.tile([128, 512], F32R, tag=f"xa{k}", name=f"xa{k}")
                    for k in range(KC)]
            xT_b = [mpool.tile([128, 512], F32R, tag=f"xb{k}", name=f"xb{k}")
                    for k in range(KC)]

            # W_q/W_k packed per head pair: tile[:, k, 0:128] = q chunk cols,
            # tile[:, k, 128:256] = k chunk cols. One contiguous DMA per pair.
            w_t = {}

            def load_w(pm):
                t = wpool.tile([128, KC * 256], F32R, tag="w", name=f"wp{pm}")
                nc.sync.dma_start(t[:], wp_d[pm])
                w_t[pm] = t

            # qkT chunks from a recycled pool (live: current + next pair)
            qkt = {}

            load_w(0)
            for k in range(KC):
                nc.sync.dma_start(xT_a[k][:], xa_d[k])
                nc.sync.dma_start(xT_b[k][:], xb_d[k])

            with tc.tile_pool(name="wv", bufs=1) as wvpool:
                w_v = [wvpool.tile([128, DMODEL], F32R, tag=f"wv{k}", name=f"wv{k}")
                       for k in range(KC)]
                for k in range(KC):
                    nc.sync.dma_start(w_v[k][:], wv_d[k])
                # remaining W_q/W_k pair blocks, in consumption order
                for m in range(1, KC):
                    load_w(m)

                xhalf = [xT_a, xT_b]

                def qk_chunk(mm, n):
                    if n == 0:
                        qkt[mm] = qkpool.tile(
                            [128, NSEQ], F32R, tag="qkt", name=f"qkt{mm}")
                    ps = mps.tile([128, 512], F32, tag="mps", name="ps_qk")
                    off = 0 if mm < KC else 128
                    w3 = w_t[mm % KC].rearrange("p (k c) -> p k c", c=256)
                    for k in range(KC):
                        nc.tensor.matmul(
                            ps[:],
                            lhsT=w3[:, k, off:off + 128],
                            rhs=xhalf[n][k][:],
                            start=(k == 0), stop=(k == KC - 1),
                        )
                    nc.vector.tensor_scalar_add(
                        qkt[mm][:, n * 512:(n + 1) * 512], ps[:], b_qk[:, mm:mm + 1],
                    )

                def v_chunk(mi, n0, nw):
                    ps = mps.tile([128, 512], F32, tag="mps", name="ps_v")
                    xh = xhalf[mi // 4]
                    c0 = (mi % 4) * 128
                    for k in range(KC):
                        nc.tensor.matmul(
                            ps[:, :nw],
                            lhsT=xh[k][:, c0:c0 + 128],
                            rhs=w_v[k][:, n0:n0 + nw],
                            start=(k == 0), stop=(with_bias is False and k == KC - 1),
                        )
                    if with_bias:
                        nc.tensor.matmul(
                            ps[:, :nw], lhsT=ones1[:, :],
                            rhs=b_v[:, n0:n0 + nw], start=False, stop=True,
                        )
                    nh = nw // DH
                    h0 = n0 // DH
                    src = ps[:, :nw].rearrange("p (h c) -> p h c", c=DH)
                    dst3 = v_ext[mi].rearrange("p (h c) -> p h c", c=VB)
                    hi = dst3[:, h0:h0 + nh, 0:DH]
                    lo = dst3[:, h0:h0 + nh, DH + 1:DH + 1 + DH]
                    nc.vector.tensor_copy(hi, src)
                    nc.vector.tensor_sub(lo, src, hi)

                for mi in range(MI):
                    d3 = v_ext[mi].rearrange("p (h c) -> p h c", c=VB)
                    nc.vector.memset(d3[:, :, DH:DH + 1], 1.0)
                    nc.vector.memset(d3[:, :, VB - 1:VB], 0.0)

                pvq = deque()  # deferred PV i-steps: (head, i, E tiles)

                def pv_step(h, i, E0, E1, tail=False):
                    # E0[j] = [A cols 0:512 | B cols 0:512] of S^T row-block j,
                    # E1[j] = the 512:1024 column halves
                    off = 512 * (h % 2)
                    Ei = E0 if i < 4 else E1
                    c0 = off + (i % 4) * 128
                    if tail:
                        # S slabs are dead in the tail: use their pool for
                        # deeper psum rotation
                        pv = sps.tile([128, NSEQ], F32, tag="sps", name="pvt")
                    else:
                        pv = mps.tile([128, VB], F32, tag="mps", name="pv")
                    for j in range(MI):
                        nc.tensor.matmul(
                            pv[:, :VB],
                            lhsT=Ei[j][:, c0:c0 + 128],
                            rhs=v_ext[j][:, h * VB:(h + 1) * VB],
                            start=(j == 0), stop=(j == MI - 1),
                        )
                    r = stpool.tile([128, 1], F32, tag="r", name="r")
                    nc.vector.reciprocal(r[:], pv[:, DH:DH + 1])
                    u = stpool.tile([128, DH], F32, tag="u", name="u")
                    if tail:
                        # ScalarE is idle after the last exp: offload the scale
                        nc.scalar.activation(
                            u[:], pv[:, 0:DH],
                            mybir.ActivationFunctionType.Copy, scale=r[:])
                    else:
                        nc.vector.tensor_scalar(
                            u[:], pv[:, 0:DH], r[:], None, op0=MUL)
                    o = stpool.tile([128, DH], F32, tag="o", name="o")
                    nc.vector.scalar_tensor_tensor(
                        o[:], pv[:, DH + 1:DH + 1 + DH], r[:], u[:],
                        op0=MUL, op1=ADD)
                    nc.sync.dma_start(
                        out_d[i * 128:(i + 1) * 128, h * DH:(h + 1) * DH], o[:],
                    )

                # prologue: n=0 halves first so exp(ps0) can start earliest
                for n in range(2):
                    for mm in (0, KC):
                        qk_chunk(mm, n)

                vq = deque((mi, n0, nw) for mi in range(MI)
                           for n0, nw in ((0, 512), (512, 256)))

                for pm in range(H // 2):
                    hA, hB = 2 * pm, 2 * pm + 1
                    q_t, k_t = qkt[pm], qkt[KC + pm]
                    EA, EB = [], []
                    nxt = []
                    if pm + 1 < H // 2:
                        nxt = [(pm + 1, 0), (pm + 1, 1),
                               (KC + pm + 1, 0), (KC + pm + 1, 1)]
                    for j in range(MI):
                        # S j-step: A and B share each slab (A -> left bank,
                        # B -> right bank) so one exp releases both heads'
                        # next matmuls and the row-tiled pair runs in parallel
                        ps0 = sps.tile([128, NSEQ], F32, tag="sps", name="ps0")
                        ps1 = sps.tile([128, NSEQ], F32, tag="sps", name="ps1")
                        for n, psn in ((0, ps0), (1, ps1)):
                            nc.tensor.matmul(
                                psn[:, 0:512],
                                lhsT=k_t[0:64, j * 128:(j + 1) * 128],
                                rhs=q_t[0:64, n * 512:(n + 1) * 512],
                                start=True, stop=True, tile_position=(0, 0),
                            )
                            nc.tensor.matmul(
                                psn[:, 512:1024],
                                lhsT=k_t[64:128, j * 128:(j + 1) * 128],
                                rhs=q_t[64:128, n * 512:(n + 1) * 512],
                                start=True, stop=True, tile_position=(64, 0),
                            )
                        e0 = epool.tile([128, NSEQ], BF16, tag="e", name="e0")
                        e1 = epool.tile([128, NSEQ], BF16, tag="e", name="e1")
                        nc.scalar.activation(e0[:], ps0[:], EXP)
                        nc.scalar.activation(e1[:], ps1[:], EXP)
                        EA.append(e0)
                        EB.append(e1)
                        # fill work after the S pair: lower scheduler priority,
                        # so it runs only while S matmuls are stalled
                        if pm == 0:
                            for _ in range(2):
                                if vq:
                                    v_chunk(*vq.popleft())
                        if j % 2 == 0 and nxt:
                            qk_chunk(*nxt.pop(0))
                        for _ in range(2):
                            if pvq:
                                pv_step(*pvq.popleft())
                    pvq.extend((hA, i, EA, EB) for i in range(MI))
                    pvq.extend((hB, i, EA, EB) for i in range(MI))
                while pvq:
                    pv_step(*pvq.popleft(), tail=True)

    nc.compile()
    _NC_CACHE[key] = nc
    return nc


def make_in_maps(x, W_qkv, b_qkv):
    x = np.asarray(x, dtype=np.float32)
    W_qkv = np.asarray(W_qkv, dtype=np.float32)
    b_qkv = np.asarray(b_qkv, dtype=np.float32)
    xT = x.transpose(0, 2, 1)                                # (B, 768, 1024)
    xa = np.ascontiguousarray(
        xT[:, :, 0:512].reshape(N_CORES, KC, 128, 512))
    xb = np.ascontiguousarray(
        xT[:, :, 512:1024].reshape(N_CORES, KC, 128, 512))
    # wp[pm] = [128 part, KC, 256] with q-chunk cols then k-chunk cols
    wr = W_qkv.reshape(KC, 128, C3)
    blocks = []
    for pm in range(KC):
        qp = wr[:, :, pm * 128:(pm + 1) * 128]               # (KC, 128, 128)
        kp = wr[:, :, DMODEL + pm * 128:DMODEL + (pm + 1) * 128]
        blocks.append(np.concatenate([qp, kp], axis=2)       # (KC, 128, 256)
                      .transpose(1, 0, 2))                   # (128, KC, 256)
    wp = np.ascontiguousarray(
        np.stack(blocks).reshape(KC, 128, KC * 256))
    wv = np.ascontiguousarray(wr[:, :, 2 * DMODEL:C3])       # (KC, 128, 768)
    b_qk = np.ascontiguousarray(
        b_qkv[:2 * DMODEL].reshape(2 * KC, 128).T)           # (128, 12)
    b_v = np.ascontiguousarray(b_qkv[2 * DMODEL:].reshape(1, DMODEL))
    ones_in = np.ones((1, 128), dtype=np.float32)
    return [
        {"xa": xa[c], "xb": xb[c], "wp": wp, "wv": wv,
         "b_qk": b_qk, "b_v": b_v, "ones_in": ones_in}
        for c in range(N_CORES)
    ]


def run(in_maps, trace=False, trace_cores=None, with_bias=True):
    nc = build_nc(with_bias=with_bias)
    return run_bass_kernel_spmd(
        nc, in_maps, list(range(N_CORES)), trace=trace, trace_cores=trace_cores,
    )


def kernel(x, W_qkv, b_qkv):
    with_bias = bool(np.any(np.asarray(b_qkv)))
    res = run(make_in_maps(x, W_qkv, b_qkv), with_bias=with_bias)
    return np.stack([res.results[c]["out"] for c in range(N_CORES)]).astype(np.float32)


# revision 23
# speedup vs baseline: 1.1334x; 1.0363x over previous
"""Multi-head attention (B=8, N=1024, D=768, H=12) on 8 TRN2 NeuronCores.

Sharding: data-parallel over batch — core b computes batch element b.

Per-core kernel (all shapes hardcoded):
  inputs (host-prepped):
    xT   (768, 1024) f32  = x[b].T
    w    (768, 2304) f32  = W_qkv
    b_qk (128, 12)   f32  = b_qkv[:1536] laid out [partition, chunk]
    b_v  (1, 768)    f32  = b_qkv[1536:]
  output:
    out  (1024, 768) f32

Pipeline (single fully-interleaved phase to keep TensorE dense and the
HAM clock at 8/8):
  - qkT chunks ((x @ W_qk + b)^T, fp32r matmuls) are produced one head
    pair ahead of the S matmuls that consume them.
  - v = x @ W_v + b_v is computed during the first pair's S steps and
    stored per head as bf16 [v_hi 64 | ones 1 | v_lo 64 | pad 1] blocks
    (hi/lo split keeps v exact; ones column gives the softmax denominator).
  - S^T[j,i] = k @ q^T per head pair via row-tiled K=64 matmuls (two heads
    concurrently on PE row groups 0:64 / 64:128).
  - E = exp(S^T) -> bf16 on ScalarE straight from PSUM (no max subtraction:
    logits are bounded ~ +-50 here, exp stays in f32/bf16 range).
  - PV: psum[i-block, 0:130] = sum_j E^T[j] @ v_block (bf16, FWL); PV
    i-steps of the previous pair fill PE slack between S j-steps.
  - epilogue per (head, i): out = (hi + lo) * recip(denom) on DVE, DMA out.
"""

from collections import deque

import numpy as np

import concourse.bass as bass
import concourse.mybir as mybir
import concourse.tile as tile
from concourse import bacc
from concourse.bass_utils import run_bass_kernel_spmd

N_CORES = 8
NSEQ = 1024
DMODEL = 768
H = 12
DH = 64
C3 = 3 * DMODEL
KC = DMODEL // 128   # 6 contraction chunks
MI = NSEQ // 128     # 8 sequence chunks
VB = 2 * DH + 2      # 130: per-head v block [hi 64 | ones 1 | lo 64 | pad 1]

F32 = mybir.dt.float32
F32R = mybir.dt.float32r
BF16 = mybir.dt.bfloat16
EXP = mybir.ActivationFunctionType.Exp
MUL = mybir.AluOpType.mult
ADD = mybir.AluOpType.add

_NC_CACHE = {}


def build_nc(with_bias=True):
    key = ("nc", with_bias)
    if key in _NC_CACHE:
        return _NC_CACHE[key]
    nc = bacc.Bacc("TRN2", target_bir_lowering=False, debug=False)
    xa_d = nc.dram_tensor("xa", [KC, 128, 512], F32R, kind="ExternalInput")
    xb_d = nc.dram_tensor("xb", [KC, 128, 512], F32R, kind="ExternalInput")
    wp_d = nc.dram_tensor("wp", [KC, 128, KC * 256], F32R, kind="ExternalInput")
    wv_d = nc.dram_tensor("wv", [KC, 128, DMODEL], F32R, kind="ExternalInput")
    bqk_d = nc.dram_tensor("b_qk", [128, 2 * KC], F32, kind="ExternalInput")
    bv_d = nc.dram_tensor("b_v", [1, DMODEL], F32R, kind="ExternalInput")
    ones_d = nc.dram_tensor("ones_in", [1, 128], F32R, kind="ExternalInput")
    out_d = nc.dram_tensor("out", [NSEQ, DMODEL], F32, kind="ExternalOutput")

    with tile.TileContext(nc) as tc:
        with (
            tc.tile_pool(name="const", bufs=1) as cpool,
            tc.tile_pool(name="main", bufs=1) as mpool,
            tc.tile_pool(name="stage", bufs=8) as stpool,
            tc.tile_pool(name="e", bufs=34) as epool,
            tc.tile_pool(name="wt", bufs=3) as wpool,
            tc.tile_pool(name="qkt", bufs=8) as qkpool,
            tc.tile_pool(name="s_ps", bufs=3, space="PSUM") as sps,
            tc.tile_pool(name="mix_ps", bufs=2, space="PSUM") as mps,
        ):
            b_qk = cpool.tile([128, 2 * KC], F32, tag="bqk")
            nc.sync.dma_start(b_qk[:], bqk_d[:])
            b_v = cpool.tile([1, DMODEL], F32R, tag="bv")
            nc.sync.dma_start(b_v[:], bv_d[:])
            ones1 = cpool.tile([1, 128], F32R, tag="ones")
            nc.sync.dma_start(ones1[:], ones_d[:])

            # persistent activations
            v_ext = [mpool.tile([128, H * VB], BF16, tag=f"vx{j}", name=f"vx{j}")
                     for j in range(MI)]
            # x^T halves, per k-chunk
            xT_a = [mpool.tile([128, 512], F32R, tag=f"xa{k}", name=f"xa{k}")
                    for k in range(KC)]
            xT_b = [mpool.tile([128, 512], F32R, tag=f"xb{k}", name=f"xb{k}")
                    for k in range(KC)]

            # W_q/W_k packed per head pair: tile[:, k, 0:128] = q chunk cols,
            # tile[:, k, 128:256] = k chunk cols. One contiguous DMA per pair.
            w_t = {}

            def load_w(pm):
                t = wpool.tile([128, KC * 256], F32R, tag="w", name=f"wp{pm}")
                nc.sync.dma_start(t[:], wp_d[pm])
                w_t[pm] = t

            # qkT chunks from a recycled pool (live: current + next pair)
            qkt = {}

            load_w(0)
            for k in range(KC):
                nc.sync.dma_start(xT_a[k][:], xa_d[k])
                nc.sync.dma_start(xT_b[k][:], xb_d[k])

            with tc.tile_pool(name="wv", bufs=1) as wvpool:
                w_v = [wvpool.tile([128, DMODEL], F32R, tag=f"wv{k}", name=f"wv{k}")
                       for k in range(KC)]
                for k in range(KC):
                    nc.sync.dma_start(w_v[k][:], wv_d[k])
                # remaining W_q/W_k pair blocks, in consumption order
                for m in range(1, KC):
                    load_w(m)

                xhalf = [xT_a, xT_b]

                def qk_chunk(mm, n):
                    if n == 0:
                        qkt[mm] = qkpool.tile(
                            [128, NSEQ], F32R, tag="qkt", name=f"qkt{mm}")
                    ps = mps.tile([128, 512], F32, tag="mps", name="ps_qk")
                    off = 0 if mm < KC else 128
                    w3 = w_t[mm % KC].rearrange("p (k c) -> p k c", c=256)
                    for k in range(KC):
                        nc.tensor.matmul(
                            ps[:],
                            lhsT=w3[:, k, off:off + 128],
                            rhs=xhalf[n][k][:],
                            start=(k == 0), stop=(k == KC - 1),
                        )
                    nc.vector.tensor_scalar_add(
                        qkt[mm][:, n * 512:(n + 1) * 512], ps[:], b_qk[:, mm:mm + 1],
                    )

                def v_chunk(mi, n0, nw):
                    ps = mps.tile([128, 512], F32, tag="mps", name="ps_v")
                    xh = xhalf[mi // 4]
                    c0 = (mi % 4) * 128
                    for k in range(KC):
                        nc.tensor.matmul(
                            ps[:, :nw],
                            lhsT=xh[k][:, c0:c0 + 128],
                            rhs=w_v[k][:, n0:n0 + nw],
                            start=(k == 0), stop=(with_bias is False and k == KC - 1),
                        )
                    if with_bias:
                        nc.tensor.matmul(
                            ps[:, :nw], lhsT=ones1[:, :],
                            rhs=b_v[:, n0:n0 + nw], start=False, stop=True,
                        )
                    nh = nw // DH
                    h0 = n0 // DH
                    src = ps[:, :nw].rearrange("p (h c) -> p h c", c=DH)
                    dst3 = v_ext[mi].rearrange("p (h c) -> p h c", c=VB)
                    hi = dst3[:, h0:h0 + nh, 0:DH]
                    lo = dst3[:, h0:h0 + nh, DH + 1:DH + 1 + DH]
                    nc.vector.tensor_copy(hi, src)
                    nc.vector.tensor_sub(lo, src, hi)

                for mi in range(MI):
                    d3 = v_ext[mi].rearrange("p (h c) -> p h c", c=VB)
                    nc.vector.memset(d3[:, :, DH:DH + 1], 1.0)
                    nc.vector.memset(d3[:, :, VB - 1:VB], 0.0)

                pvq = deque()  # deferred PV i-steps: (head, i, E tiles)

                def pv_step(h, i, E0, E1, tail=False):
                    # E0[j] = [A cols 0:512 | B cols 0:512] of S^T row-block j,
                    # E1[j] = the 512:1024 column halves
                    off = 512 * (h % 2)
                    Ei = E0 if i < 4 else E1
                    c0 = off + (i % 4) * 128
                    if tail:
                        # S slabs are dead in the tail: use their pool for
                        # deeper psum rotation
                        pv = sps.tile([128, NSEQ], F32, tag="sps", name="pvt")
                    else:
                        pv = mps.tile([128, VB], F32, tag="mps", name="pv")
                    for j in range(MI):
                        nc.tensor.matmul(
                            pv[:, :VB],
                            lhsT=Ei[j][:, c0:c0 + 128],
                            rhs=v_ext[j][:, h * VB:(h + 1) * VB],
                            start=(j == 0), stop=(j == MI - 1),
                        )
                    r = stpool.tile([128, 1], F32, tag="r", name="r")
                    nc.vector.reciprocal(r[:], pv[:, DH:DH + 1])
                    u = stpool.tile([128, DH], F32, tag="u", name="u")
                    if tail:
                        # ScalarE is idle after the last exp: offload the scale
                        nc.scalar.activation(
                            u[:], pv[:, 0:DH],
                            mybir.ActivationFunctionType.Copy, scale=r[:])
                    else:
                        nc.vector.tensor_scalar(
                            u[:], pv[:, 0:DH], r[:], None, op0=MUL)
                    o = stpool.tile([128, DH], F32, tag="o", name="o")
                    nc.vector.scalar_tensor_tensor(
                        o[:], pv[:, DH + 1:DH + 1 + DH], r[:], u[:],
                        op0=MUL, op1=ADD)
                    nc.sync.dma_start(
                        out_d[i * 128:(i + 1) * 128, h * DH:(h + 1) * DH], o[:],
                    )

                # prologue: n=0 halves first so exp(ps0) can start earliest
                for n in range(2):
                    for mm in (0, KC):
                        qk_chunk(mm, n)

                vq = deque((mi, n0, nw) for mi in range(MI)
                           for n0, nw in ((0, 512), (512, 256)))

                for pm in range(H // 2):
                    hA, hB = 2 * pm, 2 * pm + 1
                    q_t, k_t = qkt[pm], qkt[KC + pm]
                    EA, EB = [], []
                    nxt = []
                    if pm + 1 < H // 2:
                        nxt = [(pm + 1, 0), (pm + 1, 1),
                               (KC + pm + 1, 0), (KC + pm + 1, 1)]
                    for j in range(MI):
                        # S j-step: A and B share each slab (A -> left bank,
                        # B -> right bank) so one exp releases both heads'
                        # next matmuls and the row-tiled pair runs in parallel
                        ps0 = sps.tile([128, NSEQ], F32, tag="sps", name="ps0")
                        ps1 = sps.tile([128, NSEQ], F32, tag="sps", name="ps1")
                        for n, psn in ((0, ps0), (1, ps1)):
                            nc.tensor.matmul(
                                psn[:, 0:512],
                                lhsT=k_t[0:64, j * 128:(j + 1) * 128],
                                rhs=q_t[0:64, n * 512:(n + 1) * 512],
                                start=True, stop=True, tile_position=(0, 0),
                            )
                            nc.tensor.matmul(
                                psn[:, 512:1024],
                                lhsT=k_t[64:128, j * 128:(j + 1) * 128],
                                rhs=q_t[64:128, n * 512:(n + 1) * 512],
                                start=True, stop=True, tile_position=(64, 0),
                            )
                        e0 = epool.tile([128, NSEQ], BF16, tag="e", name="e0")
                        e1 = epool.tile([128, NSEQ], BF16, tag="e", name="e1")
                        nc.scalar.activation(e0[:], ps0[:], EXP)
                        nc.scalar.activation(e1[:], ps1[:], EXP)
                        EA.append(e0)
                        EB.append(e1)
                        # fill work after the S pair: lower scheduler priority,
                        # so it runs only while S matmuls are stalled
                        if pm == 0:
                            for _ in range(2):
                                if vq:
                                    v_chunk(*vq.popleft())
                        if j % 2 == 0 and nxt:
                            qk_chunk(*nxt.pop(0))
                        for _ in range(3 if j >= 6 else 2):
                            if pvq:
                                pv_step(*pvq.popleft())
                    pvq.extend((hA, i, EA, EB) for i in range(MI))
                    pvq.extend((hB, i, EA, EB) for i in range(MI))
                while pvq:
                    pv_step(*pvq.popleft(), tail=True)

    nc.compile()
    _NC_CACHE[key] = nc
    return nc


def make_in_maps(x, W_qkv, b_qkv):
    x = np.asarray(x, dtype=np.float32)
    W_qkv = np.asarray(W_qkv, dtype=np.float32)
    b_qkv = np.asarray(b_qkv, dtype=np.float32)
    xT = x.transpose(0, 2, 1)                                # (B, 768, 1024)
    xa = np.ascontiguousarray(
        xT[:, :, 0:512].reshape(N_CORES, KC, 128, 512))
    xb = np.ascontiguousarray(
        xT[:, :, 512:1024].reshape(N_CORES, KC, 128, 512))
    # wp[pm] = [128 part, KC, 256] with q-chunk cols then k-chunk cols
    wr = W_qkv.reshape(KC, 128, C3)
    blocks = []
    for pm in range(KC):
        qp = wr[:, :, pm * 128:(pm + 1) * 128]               # (KC, 128, 128)
        kp = wr[:, :, DMODEL + pm * 128:DMODEL + (pm + 1) * 128]
        blocks.append(np.concatenate([qp, kp], axis=2)       # (KC, 128, 256)
                      .transpose(1, 0, 2))                   # (128, KC, 256)
    wp = np.ascontiguousarray(
        np.stack(blocks).reshape(KC, 128, KC * 256))
    wv = np.ascontiguousarray(wr[:, :, 2 * DMODEL:C3])       # (KC, 128, 768)
    b_qk = np.ascontiguousarray(
        b_qkv[:2 * DMODEL].reshape(2 * KC, 128).T)           # (128, 12)
    b_v = np.ascontiguousarray(b_qkv[2 * DMODEL:].reshape(1, DMODEL))
    ones_in = np.ones((1, 128), dtype=np.float32)
    return [
        {"xa": xa[c], "xb": xb[c], "wp": wp, "wv": wv,
         "b_qk": b_qk, "b_v": b_v, "ones_in": ones_in}
        for c in range(N_CORES)
    ]


def run(in_maps, trace=False, trace_cores=None, with_bias=True):
    nc = build_nc(with_bias=with_bias)
    return run_bass_kernel_spmd(
        nc, in_maps, list(range(N_CORES)), trace=trace, trace_cores=trace_cores,
    )


def kernel(x, W_qkv, b_qkv):
    with_bias = bool(np.any(np.asarray(b_qkv)))
    res = run(make_in_maps(x, W_qkv, b_qkv), with_bias=with_bias)
    return np.stack([res.results[c]["out"] for c in range(N_CORES)]).astype(np.float32)


# revision 24
# speedup vs baseline: 1.1336x; 1.0002x over previous
"""Multi-head attention (B=8, N=1024, D=768, H=12) on 8 TRN2 NeuronCores.

Sharding: data-parallel over batch — core b computes batch element b.

Per-core kernel (all shapes hardcoded):
  inputs (host-prepped):
    xT   (768, 1024) f32  = x[b].T
    w    (768, 2304) f32  = W_qkv
    b_qk (128, 12)   f32  = b_qkv[:1536] laid out [partition, chunk]
    b_v  (1, 768)    f32  = b_qkv[1536:]
  output:
    out  (1024, 768) f32

Pipeline (single fully-interleaved phase to keep TensorE dense and the
HAM clock at 8/8):
  - qkT chunks ((x @ W_qk + b)^T, fp32r matmuls) are produced one head
    pair ahead of the S matmuls that consume them.
  - v = x @ W_v + b_v is computed during the first pair's S steps and
    stored per head as bf16 [v_hi 64 | ones 1 | v_lo 64 | pad 1] blocks
    (hi/lo split keeps v exact; ones column gives the softmax denominator).
  - S^T[j,i] = k @ q^T per head pair via row-tiled K=64 matmuls (two heads
    concurrently on PE row groups 0:64 / 64:128).
  - E = exp(S^T) -> bf16 on ScalarE straight from PSUM (no max subtraction:
    logits are bounded ~ +-50 here, exp stays in f32/bf16 range).
  - PV: psum[i-block, 0:130] = sum_j E^T[j] @ v_block (bf16, FWL); PV
    i-steps of the previous pair fill PE slack between S j-steps.
  - epilogue per (head, i): out = (hi + lo) * recip(denom) on DVE, DMA out.
"""

from collections import deque

import numpy as np

import concourse.bass as bass
import concourse.mybir as mybir
import concourse.tile as tile
from concourse import bacc
from concourse.bass_utils import run_bass_kernel_spmd

N_CORES = 8
NSEQ = 1024
DMODEL = 768
H = 12
DH = 64
C3 = 3 * DMODEL
KC = DMODEL // 128   # 6 contraction chunks
MI = NSEQ // 128     # 8 sequence chunks
VB = 2 * DH + 2      # 130: per-head v block [hi 64 | ones 1 | lo 64 | pad 1]

F32 = mybir.dt.float32
F32R = mybir.dt.float32r
BF16 = mybir.dt.bfloat16
EXP = mybir.ActivationFunctionType.Exp
MUL = mybir.AluOpType.mult
ADD = mybir.AluOpType.add

_NC_CACHE = {}


def build_nc(with_bias=True):
    key = ("nc", with_bias)
    if key in _NC_CACHE:
        return _NC_CACHE[key]
    nc = bacc.Bacc("TRN2", target_bir_lowering=False, debug=False)
    xa_d = nc.dram_tensor("xa", [KC, 128, 512], F32R, kind="ExternalInput")
    xb_d = nc.dram_tensor("xb", [KC, 128, 512], F32R, kind="ExternalInput")
    wp_d = nc.dram_tensor("wp", [KC, 128, KC * 256], F32R, kind="ExternalInput")
    wv_d = nc.dram_tensor("wv", [KC, 128, DMODEL], F32R, kind="ExternalInput")
    bqk_d = nc.dram_tensor("b_qk", [128, 2 * KC], F32, kind="ExternalInput")
    bv_d = nc.dram_tensor("b_v", [1, DMODEL], F32R, kind="ExternalInput")
    ones_d = nc.dram_tensor("ones_in", [1, 128], F32R, kind="ExternalInput")
    out_d = nc.dram_tensor("out", [NSEQ, DMODEL], F32, kind="ExternalOutput")

    with tile.TileContext(nc) as tc:
        with (
            tc.tile_pool(name="const", bufs=1) as cpool,
            tc.tile_pool(name="main", bufs=1) as mpool,
            tc.tile_pool(name="stage", bufs=12) as stpool,
            tc.tile_pool(name="e", bufs=34) as epool,
            tc.tile_pool(name="wt", bufs=3) as wpool,
            tc.tile_pool(name="qkt", bufs=8) as qkpool,
            tc.tile_pool(name="s_ps", bufs=3, space="PSUM") as sps,
            tc.tile_pool(name="mix_ps", bufs=2, space="PSUM") as mps,
        ):
            b_qk = cpool.tile([128, 2 * KC], F32, tag="bqk")
            nc.sync.dma_start(b_qk[:], bqk_d[:])
            b_v = cpool.tile([1, DMODEL], F32R, tag="bv")
            nc.sync.dma_start(b_v[:], bv_d[:])
            ones1 = cpool.tile([1, 128], F32R, tag="ones")
            nc.sync.dma_start(ones1[:], ones_d[:])

            # persistent activations
            v_ext = [mpool.tile([128, H * VB], BF16, tag=f"vx{j}", name=f"vx{j}")
                     for j in range(MI)]
            # x^T halves, per k-chunk
            xT_a = [mpool.tile([128, 512], F32R, tag=f"xa{k}", name=f"xa{k}")
                    for k in range(KC)]
            xT_b = [mpool.tile([128, 512], F32R, tag=f"xb{k}", name=f"xb{k}")
                    for k in range(KC)]

            # W_q/W_k packed per head pair: tile[:, k, 0:128] = q chunk cols,
            # tile[:, k, 128:256] = k chunk cols. One contiguous DMA per pair.
            w_t = {}

            def load_w(pm):
                t = wpool.tile([128, KC * 256], F32R, tag="w", name=f"wp{pm}")
                nc.sync.dma_start(t[:], wp_d[pm])
                w_t[pm] = t

            # qkT chunks from a recycled pool (live: current + next pair)
            qkt = {}

            load_w(0)
            for k in range(KC):
                nc.sync.dma_start(xT_a[k][:], xa_d[k])
            for k in range(KC):
                nc.sync.dma_start(xT_b[k][:], xb_d[k])

            with tc.tile_pool(name="wv", bufs=1) as wvpool:
                w_v = [wvpool.tile([128, DMODEL], F32R, tag=f"wv{k}", name=f"wv{k}")
                       for k in range(KC)]
                for k in range(KC):
                    nc.sync.dma_start(w_v[k][:], wv_d[k])
                # remaining W_q/W_k pair blocks, in consumption order
                for m in range(1, KC):
                    load_w(m)

                xhalf = [xT_a, xT_b]

                def qk_chunk(mm, n):
                    if n == 0:
                        qkt[mm] = qkpool.tile(
                            [128, NSEQ], F32R, tag="qkt", name=f"qkt{mm}")
                    ps = mps.tile([128, 512], F32, tag="mps", name="ps_qk")
                    off = 0 if mm < KC else 128
                    w3 = w_t[mm % KC].rearrange("p (k c) -> p k c", c=256)
                    for k in range(KC):
                        nc.tensor.matmul(
                            ps[:],
                            lhsT=w3[:, k, off:off + 128],
                            rhs=xhalf[n][k][:],
                            start=(k == 0), stop=(k == KC - 1),
                        )
                    nc.vector.tensor_scalar_add(
                        qkt[mm][:, n * 512:(n + 1) * 512], ps[:], b_qk[:, mm:mm + 1],
                    )

                def v_chunk(mi, n0, nw):
                    ps = mps.tile([128, 512], F32, tag="mps", name="ps_v")
                    xh = xhalf[mi // 4]
                    c0 = (mi % 4) * 128
                    for k in range(KC):
                        nc.tensor.matmul(
                            ps[:, :nw],
                            lhsT=xh[k][:, c0:c0 + 128],
                            rhs=w_v[k][:, n0:n0 + nw],
                            start=(k == 0), stop=(with_bias is False and k == KC - 1),
                        )
                    if with_bias:
                        nc.tensor.matmul(
                            ps[:, :nw], lhsT=ones1[:, :],
                            rhs=b_v[:, n0:n0 + nw], start=False, stop=True,
                        )
                    nh = nw // DH
                    h0 = n0 // DH
                    src = ps[:, :nw].rearrange("p (h c) -> p h c", c=DH)
                    dst3 = v_ext[mi].rearrange("p (h c) -> p h c", c=VB)
                    hi = dst3[:, h0:h0 + nh, 0:DH]
                    lo = dst3[:, h0:h0 + nh, DH + 1:DH + 1 + DH]
                    nc.vector.tensor_copy(hi, src)
                    nc.vector.tensor_sub(lo, src, hi)

                for mi in range(MI):
                    d3 = v_ext[mi].rearrange("p (h c) -> p h c", c=VB)
                    nc.vector.memset(d3[:, :, DH:DH + 1], 1.0)
                    nc.vector.memset(d3[:, :, VB - 1:VB], 0.0)

                pvq = deque()  # deferred PV i-steps: (head, i, E tiles)

                def pv_step(h, i, E0, E1, tail=False):
                    # E0[j] = [A cols 0:512 | B cols 0:512] of S^T row-block j,
                    # E1[j] = the 512:1024 column halves
                    off = 512 * (h % 2)
                    Ei = E0 if i < 4 else E1
                    c0 = off + (i % 4) * 128
                    if tail:
                        # S slabs are dead in the tail: use their pool for
                        # deeper psum rotation
                        pv = sps.tile([128, NSEQ], F32, tag="sps", name="pvt")
                    else:
                        pv = mps.tile([128, VB], F32, tag="mps", name="pv")
                    for j in range(MI):
                        nc.tensor.matmul(
                            pv[:, :VB],
                            lhsT=Ei[j][:, c0:c0 + 128],
                            rhs=v_ext[j][:, h * VB:(h + 1) * VB],
                            start=(j == 0), stop=(j == MI - 1),
                        )
                    r = stpool.tile([128, 1], F32, tag="r", name="r")
                    nc.vector.reciprocal(r[:], pv[:, DH:DH + 1])
                    u = stpool.tile([128, DH], F32, tag="u", name="u")
                    if tail:
                        # ScalarE is idle after the last exp: offload the scale
                        nc.scalar.activation(
                            u[:], pv[:, 0:DH],
                            mybir.ActivationFunctionType.Copy, scale=r[:])
                    else:
                        nc.vector.tensor_scalar(
                            u[:], pv[:, 0:DH], r[:], None, op0=MUL)
                    o = stpool.tile([128, DH], F32, tag="o", name="o")
                    nc.vector.scalar_tensor_tensor(
                        o[:], pv[:, DH + 1:DH + 1 + DH], r[:], u[:],
                        op0=MUL, op1=ADD)
                    nc.sync.dma_start(
                        out_d[i * 128:(i + 1) * 128, h * DH:(h + 1) * DH], o[:],
                    )

                # prologue: n=0 halves first so exp(ps0) can start earliest
                for n in range(2):
                    for mm in (0, KC):
                        qk_chunk(mm, n)

                vq = deque((mi, n0, nw) for mi in range(MI)
                           for n0, nw in ((0, 512), (512, 256)))

                for pm in range(H // 2):
                    hA, hB = 2 * pm, 2 * pm + 1
                    q_t, k_t = qkt[pm], qkt[KC + pm]
                    EA, EB = [], []
                    nxt = []
                    if pm + 1 < H // 2:
                        nxt = [(pm + 1, 0), (pm + 1, 1),
                               (KC + pm + 1, 0), (KC + pm + 1, 1)]
                    for j in range(MI):
                        # S j-step: A and B share each slab (A -> left bank,
                        # B -> right bank) so one exp releases both heads'
                        # next matmuls and the row-tiled pair runs in parallel
                        ps0 = sps.tile([128, NSEQ], F32, tag="sps", name="ps0")
                        ps1 = sps.tile([128, NSEQ], F32, tag="sps", name="ps1")
                        for n, psn in ((0, ps0), (1, ps1)):
                            nc.tensor.matmul(
                                psn[:, 0:512],
                                lhsT=k_t[0:64, j * 128:(j + 1) * 128],
                                rhs=q_t[0:64, n * 512:(n + 1) * 512],
                                start=True, stop=True, tile_position=(0, 0),
                            )
                            nc.tensor.matmul(
                                psn[:, 512:1024],
                                lhsT=k_t[64:128, j * 128:(j + 1) * 128],
                                rhs=q_t[64:128, n * 512:(n + 1) * 512],
                                start=True, stop=True, tile_position=(64, 0),
                            )
                        e0 = epool.tile([128, NSEQ], BF16, tag="e", name="e0")
                        e1 = epool.tile([128, NSEQ], BF16, tag="e", name="e1")
                        nc.scalar.activation(e0[:], ps0[:], EXP)
                        nc.scalar.activation(e1[:], ps1[:], EXP)
                        EA.append(e0)
                        EB.append(e1)
                        # fill work after the S pair: lower scheduler priority,
                        # so it runs only while S matmuls are stalled
                        if pm == 0:
                            for _ in range(2):
                                if vq:
                                    v_chunk(*vq.popleft())
                        if j % 2 == 0 and nxt:
                            qk_chunk(*nxt.pop(0))
                        for _ in range(3 if j >= 5 else 2):
                            if pvq:
                                pv_step(*pvq.popleft())
                    pvq.extend((hA, i, EA, EB) for i in range(MI))
                    pvq.extend((hB, i, EA, EB) for i in range(MI))
                while pvq:
                    pv_step(*pvq.popleft(), tail=True)

    nc.compile()
    _NC_CACHE[key] = nc
    return nc


def make_in_maps(x, W_qkv, b_qkv):
    x = np.asarray(x, dtype=np.float32)
    W_qkv = np.asarray(W_qkv, dtype=np.float32)
    b_qkv = np.asarray(b_qkv, dtype=np.float32)
    xT = x.transpose(0, 2, 1)                                # (B, 768, 1024)
    xa = np.ascontiguousarray(
        xT[:, :, 0:512].reshape(N_CORES, KC, 128, 512))
    xb = np.ascontiguousarray(
        xT[:, :, 512:1024].reshape(N_CORES, KC, 128, 512))
    # wp[pm] = [128 part, KC, 256] with q-chunk cols then k-chunk cols
    wr = W_qkv.reshape(KC, 128, C3)
    blocks = []
    for pm in range(KC):
        qp = wr[:, :, pm * 128:(pm + 1) * 128]               # (KC, 128, 128)
        kp = wr[:, :, DMODEL + pm * 128:DMODEL + (pm + 1) * 128]
        blocks.append(np.concatenate([qp, kp], axis=2)       # (KC, 128, 256)
                      .transpose(1, 0, 2))                   # (128, KC, 256)
    wp = np.ascontiguousarray(
        np.stack(blocks).reshape(KC, 128, KC * 256))
    wv = np.ascontiguousarray(wr[:, :, 2 * DMODEL:C3])       # (KC, 128, 768)
    b_qk = np.ascontiguousarray(
        b_qkv[:2 * DMODEL].reshape(2 * KC, 128).T)           # (128, 12)
    b_v = np.ascontiguousarray(b_qkv[2 * DMODEL:].reshape(1, DMODEL))
    ones_in = np.ones((1, 128), dtype=np.float32)
    return [
        {"xa": xa[c], "xb": xb[c], "wp": wp, "wv": wv,
         "b_qk": b_qk, "b_v": b_v, "ones_in": ones_in}
        for c in range(N_CORES)
    ]


def run(in_maps, trace=False, trace_cores=None, with_bias=True):
    nc = build_nc(with_bias=with_bias)
    return run_bass_kernel_spmd(
        nc, in_maps, list(range(N_CORES)), trace=trace, trace_cores=trace_cores,
    )


def kernel(x, W_qkv, b_qkv):
    with_bias = bool(np.any(np.asarray(b_qkv)))
    res = run(make_in_maps(x, W_qkv, b_qkv), with_bias=with_bias)
    return np.stack([res.results[c]["out"] for c in range(N_CORES)]).astype(np.float32)
